# revision 1
# baseline (speedup 1.0000x reference)
"""Multi-head attention TRN2 kernel: 8 cores = 4 batch x 2 head-groups.

Per core (b = core//2, g = core%2): computes the attention block for batch
sample b restricted to heads [8g, 8g+8), producing the (transposed) partial
output projection. Host sums the two head-group partials per batch + bias.

v4:
- K^T/Q^T stored fp16 (projections computed in f32r): the row-tiled QK
  pair shares the moving-operand stream, so 2-byte operands double QK
  throughput vs f32r; fp16's 10-bit mantissa keeps the scores accurate.
- Q^T resident in SBUF (no DRAM roundtrip).
- Attention loops q-chunk outer / pair inner with one-chunk lookahead:
  Q-projection for chunk q+1 is emitted after pair 0 of chunk q, and the
  output projection of chunk q is emitted inside chunk q+1, so the
  exp-bound steady state never runs dry at chunk boundaries.
- Normalization uses a direct [1, TC] reciprocal from PSUM + partition
  broadcast (no gather/scatter roundtrip).
- K-projection keeps each weight block stationary across the 4 token
  chunks (x resident per-db) to cut LDWEIGHTS count.

Layouts (per core, host-prepped):
  xq/xk/xv : x^T        [1024 d, 2048 t] f32/bf16
  wq/wk/wv : W_g^T      [1024 d, 512 j]  f32/bf16
  wo       : Wo_g^T     [512 c, 1024 j]  bf16
  out      : OUT^T partial [1024 j, 2048 t] f32
"""

import numpy as np
import ml_dtypes

D = 1024          # d_model
L = 2048          # sequence length
B = 4             # batch
HG = 512          # head-group width (8 heads x 64)
NCORES = 8
EXP_BIAS = -45.0  # softmax shift: exp(S-45); cancels in normalization

NT = 4            # token chunks of 512
TC = L // NT      # 512
NDB = D // 128    # 8 d-model blocks
NP = 4            # head pairs per group
NKB = L // 128    # 16 key blocks

_COMPILED = None
LAST_RESULT = None


def _build():
    import concourse.bacc as bacc
    import concourse.mybir as mybir
    import concourse.tile as tile

    f32 = mybir.dt.float32
    f32r = mybir.dt.float32r
    bf16 = mybir.dt.bfloat16
    f16 = mybir.dt.float16
    EXP = mybir.ActivationFunctionType.Exp
    ADD = mybir.AluOpType.add
    MUL = mybir.AluOpType.mult

    nc = bacc.Bacc()

    xq = nc.declare_dram_parameter("xq", [D, L], f32r, isOutput=False)
    xk = nc.declare_dram_parameter("xk", [D, L], f32r, isOutput=False)
    xv = nc.declare_dram_parameter("xv", [D, L], bf16, isOutput=False)
    wq = nc.declare_dram_parameter("wq", [D, HG], f32r, isOutput=False)
    wk = nc.declare_dram_parameter("wk", [D, HG], f32r, isOutput=False)
    wv = nc.declare_dram_parameter("wv", [D, HG], bf16, isOutput=False)
    wo = nc.declare_dram_parameter("wo", [HG, D], bf16, isOutput=False)
    bq = nc.declare_dram_parameter("bq", [HG], f32, isOutput=False)
    bv = nc.declare_dram_parameter("bv", [HG], f32, isOutput=False)
    out = nc.declare_dram_parameter("out", [D, L], f32, isOutput=True)

    out_v = out.rearrange("(ob p) (n t) -> ob p n t", p=128, t=TC)

    with tile.TileContext(nc) as tc:
        with tc.tile_pool(name="res", bufs=1) as res, tc.tile_pool(
            name="psum", bufs=1, space="PSUM"
        ) as psum, tc.tile_pool(name="pq", bufs=1) as pq:
            # ---- resident tiles ----
            kt_sb = res.tile([128, NP, L], f16)        # resident K^T (fp16)
            qt_sb = res.tile([128, NP, L], f16)        # resident Q^T (fp16)
            bq_sb = res.tile([128, NP], f32)
            bv_row = res.tile([1, HG], f32)
            bv_bc = res.tile([128, HG], f32)
            bias_exp = res.tile([128, 1], f32)
            wo_sb = res.tile([128, NP, D], bf16)

            # V in AV-stationary layout: per (kb, pair): [Vh_even, 1, Vh_odd, 1]
            v_sb = res.tile([128, NKB, NP, 130], bf16)

            # Q-proj weights + x chunks stay alive through attention (Q-proj
            # chunks t1-t3 are interleaved into attention chunks)
            wq_sb = pq.tile([128, NDB, HG], f32r)

            def load_xq_chunk(t):
                xt = pq.tile([128, NDB, TC], f32r, name="xqt", tag="xqt", bufs=1)
                nc.sync.dma_start(
                    out=xt[:],
                    in_=xq.rearrange("(db p) (n t) -> p db n t", p=128, t=TC)[
                        :, :, t
                    ],
                )
                return xt

            def qproj_chunk(t, xq_t):
                for jb in range(NP):
                    ps = psum.tile([128, TC], f32, name="acc", tag="accu", bufs=2)
                    for db in range(NDB):
                        nc.tensor.matmul(
                            ps[:],
                            wq_sb[:, db, jb * 128 : (jb + 1) * 128],
                            xq_t[:, db, :],
                            start=(db == 0),
                            stop=(db == NDB - 1),
                        )
                    nc.vector.tensor_scalar_add(
                        qt_sb[:, jb, t * TC : (t + 1) * TC],
                        ps[:],
                        bq_sb[:, jb : jb + 1],
                    )

            # ---- K / V projections (scoped pools, freed before attention) ----
            with tc.tile_pool(name="pw", bufs=1) as pw, tc.tile_pool(
                name="px", bufs=1
            ) as px:
                # K first. Split weights into a small head tile (db 0-1) and
                # the rest so the first matmuls only wait on a 512KB DMA.
                wk_a = pw.tile([128, 2, HG], f32r)
                wk_b = pw.tile([128, NDB - 2, HG], f32r)
                wv_sb = pw.tile([128, NDB, HG], bf16)
                wk_v = wk.rearrange("(db p) j -> p db j", p=128)
                nc.sync.dma_start(out=wk_a[:], in_=wk_v[:, 0:2])
                nc.sync.dma_start(out=wk_b[:], in_=wk_v[:, 2:NDB])

                def wk_blk(db, jb):
                    if db < 2:
                        return wk_a[:, db, jb * 128 : (jb + 1) * 128]
                    return wk_b[:, db - 2, jb * 128 : (jb + 1) * 128]

                # x^T for K resident as per-db tiles so each weight block
                # stays stationary across all 4 token chunks
                xk_db = [
                    px.tile([128, L], f32r, name=f"xk{db}") for db in range(NDB)
                ]
                for db in range(NDB):
                    nc.sync.dma_start(
                        out=xk_db[db][:],
                        in_=xk.rearrange("(db p) t -> p db t", p=128)[:, db],
                    )

                # K^T projection -> kt_sb (fp16); weight-stationary loop
                for jb in range(NP):
                    # borrow the attention-phase ps_s slots: 2 two-bank tiles
                    # give 4 accumulation targets (one per token chunk)
                    kp0 = psum.tile([128, 2, TC], f32, name="kp0", tag="ps_s", bufs=2)
                    kp1 = psum.tile([128, 2, TC], f32, name="kp1", tag="ps_s", bufs=2)
                    pss = [kp0[:, 0, :], kp0[:, 1, :], kp1[:, 0, :], kp1[:, 1, :]]
                    for db in range(NDB):
                        for t in range(NT):
                            nc.tensor.matmul(
                                pss[t],
                                wk_blk(db, jb),
                                xk_db[db][:, t * TC : (t + 1) * TC],
                                start=(db == 0),
                                stop=(db == NDB - 1),
                            )
                    for t in range(NT):
                        nc.vector.tensor_copy(
                            kt_sb[:, jb, t * TC : (t + 1) * TC], pss[t]
                        )
                    if jb == 0:
                        # remaining resident DMAs, after the critical K path
                        nc.sync.dma_start(
                            out=wv_sb[:],
                            in_=wv.rearrange("(db p) j -> p db j", p=128),
                        )
                        nc.sync.dma_start(
                            out=wo_sb[:],
                            in_=wo.rearrange("(cb p) j -> p cb j", p=128),
                        )
                        nc.sync.dma_start(
                            out=wq_sb[:],
                            in_=wq.rearrange("(db p) j -> p db j", p=128),
                        )
                        nc.sync.dma_start(
                            out=bq_sb[:],
                            in_=bq.rearrange("(jb p) -> p jb", p=128),
                        )
                        nc.sync.dma_start(
                            out=bv_row[:],
                            in_=bv.rearrange("(o j) -> o j", o=1),
                        )
                        nc.gpsimd.partition_broadcast(
                            bv_bc[:], bv_row[:], channels=128
                        )
                        nc.vector.memset(bias_exp[:], EXP_BIAS)
                        nc.vector.memset(v_sb[:, :, :, 64:65], 1.0)
                        nc.vector.memset(v_sb[:, :, :, 129:130], 1.0)

                # V projection (natural layout, +bias) -> v_sb
                for t in range(NT):
                    xv_t = px.tile(
                        [128, NDB, TC], bf16, name="xvt", tag="xvt", bufs=2
                    )
                    nc.sync.dma_start(
                        out=xv_t[:],
                        in_=xv.rearrange("(db p) (n t) -> p db n t", p=128, t=TC)[
                            :, :, t
                        ],
                    )
                    for tb in range(4):
                        kb = t * 4 + tb
                        ps = psum.tile([128, HG], f32, name="acc", tag="accu", bufs=2)
                        for db in range(NDB):
                            nc.tensor.matmul(
                                ps[:],
                                xv_t[:, db, tb * 128 : (tb + 1) * 128],
                                wv_sb[:, db, :],
                                start=(db == 0),
                                stop=(db == NDB - 1),
                            )
                        for p in range(NP):
                            nc.vector.tensor_tensor(
                                out=v_sb[:, kb, p, 0:64],
                                in0=ps[:, p * 128 : p * 128 + 64],
                                in1=bv_bc[:, p * 128 : p * 128 + 64],
                                op=ADD,
                            )
                            nc.vector.tensor_tensor(
                                out=v_sb[:, kb, p, 65:129],
                                in0=ps[:, p * 128 + 64 : p * 128 + 128],
                                in1=bv_bc[:, p * 128 + 64 : p * 128 + 128],
                                op=ADD,
                            )

            # Q^T projection chunk t0 (rest interleaved into attention)
            xq_t0 = load_xq_chunk(0)
            qproj_chunk(0, xq_t0)

            # ---- attention (q-chunk outer, pair inner) + lookahead emits ----
            with tc.tile_pool(name="pa", bufs=1) as pa:
                prev_ct = None  # (ct tile, q index) pending output projection

                def oproj_piece(ct_prev, q_prev, ob):
                    ps = psum.tile([128, TC], f32, name="acc", tag="ps_s", bufs=2)
                    for p in range(NP):
                        nc.tensor.matmul(
                            ps[:],
                            wo_sb[:, p, ob * 128 : (ob + 1) * 128],
                            ct_prev[:, p, :],
                            start=(p == 0),
                            stop=(p == NP - 1),
                        )
                    o_sb = pa.tile([128, TC], f32, name="o_sb", tag="o_sb", bufs=2)
                    nc.vector.tensor_copy(o_sb[:], ps[:])
                    nc.sync.dma_start(out=out_v[ob, :, q_prev], in_=o_sb[:])

                def qproj_piece(t, xq_t, jb):
                    ps = psum.tile([128, TC], f32, name="acc", tag="accu", bufs=2)
                    for db in range(NDB):
                        nc.tensor.matmul(
                            ps[:],
                            wq_sb[:, db, jb * 128 : (jb + 1) * 128],
                            xq_t[:, db, :],
                            start=(db == 0),
                            stop=(db == NDB - 1),
                        )
                    nc.vector.tensor_scalar_add(
                        qt_sb[:, jb, t * TC : (t + 1) * TC],
                        ps[:],
                        bq_sb[:, jb : jb + 1],
                    )

                for q in range(NT):
                    xq_next = load_xq_chunk(q + 1) if q + 1 < NT else None
                    # pending lookahead pieces: previous chunk's output
                    # projection + next chunk's Q projection, injected into
                    # the exp-paced idle slots of this chunk's QK phases
                    pieces = []
                    if prev_ct is not None:
                        ct_prev, q_prev = prev_ct
                        for ob in range(NDB):
                            pieces.append(
                                (lambda ob=ob, c=ct_prev, qq=q_prev:
                                 oproj_piece(c, qq, ob))
                            )
                        prev_ct = None
                    if q + 1 < NT:
                        for jb in range(NP):
                            pieces.append(
                                (lambda jb=jb, t=q + 1, x=xq_next:
                                 qproj_piece(t, x, jb))
                            )
                    pieces.reverse()  # pop() emits in original order

                    ct = pa.tile([128, NP, TC], bf16, name="ct", tag="ct", bufs=2)
                    for p in range(NP):
                        # P^T for both heads: [kb][head e/o][q]
                        pt = pa.tile(
                            [128, NKB, 2, TC], bf16, name="pt", tag="pt", bufs=2
                        )
                        for kb in range(NKB):
                            ps_s = psum.tile(
                                [128, 2, TC], f32, name="ps_s", tag="ps_s", bufs=2
                            )
                            nc.tensor.matmul(
                                ps_s[:, 0, :],
                                kt_sb[0:64, p, kb * 128 : (kb + 1) * 128],
                                qt_sb[0:64, p, q * TC : (q + 1) * TC],
                                start=True,
                                stop=True,
                            )
                            nc.tensor.matmul(
                                ps_s[:, 1, :],
                                kt_sb[64:128, p, kb * 128 : (kb + 1) * 128],
                                qt_sb[64:128, p, q * TC : (q + 1) * TC],
                                start=True,
                                stop=True,
                            )
                            nc.scalar.activation(
                                pt[:, kb, :, :], ps_s[:], EXP,
                                bias=bias_exp[:], scale=1.0,
                            )
                            if kb % 4 == 3 and kb != NKB - 1 and pieces:
                                pieces.pop()()
                        # AV: U^T + rowsum via ones column (M=65)
                        ps_u = psum.tile(
                            [128, 2, TC], f32, name="ps_u", tag="accu", bufs=2
                        )
                        for kb in range(NKB):
                            nc.tensor.matmul(
                                ps_u[0:65, 0, :],
                                v_sb[:, kb, p, 0:65],
                                pt[:, kb, 0, :],
                                start=(kb == 0),
                                stop=(kb == NKB - 1),
                            )
                            nc.tensor.matmul(
                                ps_u[0:65, 1, :],
                                v_sb[:, kb, p, 65:130],
                                pt[:, kb, 1, :],
                                start=(kb == 0),
                                stop=(kb == NKB - 1),
                            )
                        # normalize: C^T = U^T * (1/r)
                        rr_e = pa.tile([1, TC], f32, name="rr_e", tag="rr_e", bufs=2)
                        rr_o = pa.tile([1, TC], f32, name="rr_o", tag="rr_o", bufs=2)
                        nc.vector.tensor_copy(rr_e[:], ps_u[64:65, 0, :])
                        nc.vector.tensor_copy(rr_o[:], ps_u[64:65, 1, :])
                        r128 = pa.tile([128, 8], f32, name="r128", tag="r128", bufs=2)
                        nc.sync.dma_start(out=r128[:, 0:4], in_=rr_e[:])
                        nc.sync.dma_start(out=r128[:, 4:8], in_=rr_o[:])
                        nc.vector.reciprocal(r128[:], r128[:])
                        rv_e = pa.tile([1, TC], f32, name="rv_e", tag="rv_e", bufs=2)
                        rv_o = pa.tile([1, TC], f32, name="rv_o", tag="rv_o", bufs=2)
                        nc.sync.dma_start(out=rv_e[:], in_=r128[:, 0:4])
                        nc.sync.dma_start(out=rv_o[:], in_=r128[:, 4:8])
                        rb_e = pa.tile([64, TC], f32, name="rb_e", tag="rb_e", bufs=2)
                        rb_o = pa.tile([64, TC], f32, name="rb_o", tag="rb_o", bufs=2)
                        nc.gpsimd.partition_broadcast(rb_e[:], rv_e[:], channels=64)
                        nc.gpsimd.partition_broadcast(rb_o[:], rv_o[:], channels=64)
                        nc.vector.tensor_tensor(
                            out=ct[0:64, p, :],
                            in0=ps_u[0:64, 0, :],
                            in1=rb_e[:],
                            op=MUL,
                        )
                        # odd head: compute at partitions 0-63, DMA-shift to 64-127
                        ct_o = pa.tile([64, TC], bf16, name="ct_o", tag="ct_o", bufs=2)
                        nc.vector.tensor_tensor(
                            out=ct_o[:], in0=ps_u[0:64, 1, :], in1=rb_o[:], op=MUL
                        )
                        nc.sync.dma_start(out=ct[64:128, p, :], in_=ct_o[:])

                    prev_ct = (ct, q)

                # tail: output projection of the last chunk
                ct_prev, q_prev = prev_ct
                for ob in range(NDB):
                    oproj_piece(ct_prev, q_prev, ob)

    nc.compile()
    return nc


def _get_compiled():
    global _COMPILED
    if _COMPILED is None:
        _COMPILED = _build()
    return _COMPILED


def kernel(q, k, v, Wq, bq, Wk, bk, Wv, bv, Wo, bo):
    global LAST_RESULT
    from concourse.bass_utils import run_bass_kernel_spmd

    nc = _get_compiled()

    q = np.asarray(q, dtype=np.float32)
    k = np.asarray(k, dtype=np.float32)
    v = np.asarray(v, dtype=np.float32)
    Wq = np.asarray(Wq, dtype=np.float32)
    Wk = np.asarray(Wk, dtype=np.float32)
    Wv = np.asarray(Wv, dtype=np.float32)
    Wo = np.asarray(Wo, dtype=np.float32)
    bq = np.asarray(bq, dtype=np.float32)
    bv = np.asarray(bv, dtype=np.float32)
    bo = np.asarray(bo, dtype=np.float32)

    xT = {}
    for b in range(B):
        xT[("q", b)] = np.ascontiguousarray(q[b].T)
        xT[("k", b)] = np.ascontiguousarray(k[b].T)
        xT[("v", b)] = np.ascontiguousarray(v[b].T).astype(ml_dtypes.bfloat16)

    wqT = [np.ascontiguousarray(Wq[g * HG : (g + 1) * HG, :].T) for g in range(2)]
    wkT = [np.ascontiguousarray(Wk[g * HG : (g + 1) * HG, :].T) for g in range(2)]
    wvT = [
        np.ascontiguousarray(Wv[g * HG : (g + 1) * HG, :].T).astype(ml_dtypes.bfloat16)
        for g in range(2)
    ]
    woT = [
        np.ascontiguousarray(Wo[:, g * HG : (g + 1) * HG].T).astype(ml_dtypes.bfloat16)
        for g in range(2)
    ]
    bqg = [np.ascontiguousarray(bq[g * HG : (g + 1) * HG]) for g in range(2)]
    bvg = [np.ascontiguousarray(bv[g * HG : (g + 1) * HG]) for g in range(2)]

    in_maps = []
    for core in range(NCORES):
        b, g = core // 2, core % 2
        in_maps.append(
            {
                "xq": xT[("q", b)],
                "xk": xT[("k", b)],
                "xv": xT[("v", b)],
                "wq": wqT[g],
                "wk": wkT[g],
                "wv": wvT[g],
                "wo": woT[g],
                "bq": bqg[g],
                "bv": bvg[g],
            }
        )

    res = run_bass_kernel_spmd(nc, in_maps, core_ids=list(range(NCORES)))
    LAST_RESULT = res

    outp = np.empty((B, L, D), dtype=np.float32)
    for b in range(B):
        acc = res.results[2 * b]["out"].T + res.results[2 * b + 1]["out"].T
        outp[b] = acc + bo
    return outp

